# revision 42
# baseline (speedup 1.0000x reference)
"""Trainium2 Bass kernel for 16-head causal MHA with interleaved RoPE.

Problem: x (2, 2048, 1024) f32, wq/wk/wv/wo (1024, 1024) f32, positions arange(2048).
  q/k/v = x @ W.T ; RoPE(q, k) ; causal softmax attention ; out @ wo.T

Sharding: hybrid batch x head tensor-parallel over 8 cores.
  core c -> batch b = c//4, head group g = c%4 (4 heads = 256 of the 1024 c-dim).
  Each core computes its 4 heads' attention output and a partial o_proj
  (2048, 1024); a ReduceScatter over the 4 cores of each batch sums partials
  and leaves each core with a distinct 512-row slice, reassembled on host.

Device layout (per core):
  qT/kT computed transposed: (256 rows = 4 heads x 64 dk, 2048 tokens), RoPE
  applied in this layout via a PE permutation matmul + DVE/GPSIMD elementwise.
  v natural (2048, 4, 65) with a ones column -> softmax denominator comes out
  of the attention-value matmul (row 64 of each per-head PSUM accumulator).
  Scores computed transposed (tk partition, tq free) per 128-key chunk;
  exp on ACT; causal masking by zeroing post-exp via gpsimd.affine_select.
  Matmul operands are float32r (TF32-like, 1 PE cycle/row vs 4 for fp32).
"""

import sys

sys.path.insert(0, "/opt/trn_rl_repo")

import numpy as np

import concourse.bacc as bacc
import concourse.tile as tile
import concourse.mybir as mybir
from concourse.bass_utils import run_bass_kernel_spmd

B, S, D = 2, 2048, 1024
H, DK = 16, 64
NCORES = 8
HPC = 4              # heads per core
CD = HPC * DK        # 256: local c-dim (slice of D owned by one core)
TB = 512             # tq block size
NT = S // TB         # 4 query blocks
KC = D // 128        # 8 contraction chunks for the projections
THETA = 10000.0

F32 = mybir.dt.float32
F32R = mybir.dt.float32r

_CACHE = {}


def _build(collective=True, reps=1):
    nc = bacc.Bacc("TRN2", target_bir_lowering=False, debug=False,
                   num_devices=NCORES if collective else 1)

    xT_d = nc.dram_tensor("xT", [D, S], F32R, kind="ExternalInput").ap()
    wq_d = nc.dram_tensor("wqT", [D, CD], F32R, kind="ExternalInput").ap()
    wk_d = nc.dram_tensor("wkT", [D, CD], F32R, kind="ExternalInput").ap()
    wv_d = nc.dram_tensor("wvT", [D, CD], F32R, kind="ExternalInput").ap()
    wo_d = nc.dram_tensor("woT", [CD, D], F32R, kind="ExternalInput").ap()
    r2_d = nc.dram_tensor("r2T", [128, 128], F32R, kind="ExternalInput").ap()
    cos_d = nc.dram_tensor("cosT", [128, S], F32, kind="ExternalInput").ap()
    sin_d = nc.dram_tensor("sinT", [128, S], F32, kind="ExternalInput").ap()
    out_d = nc.dram_tensor("out", [NT, 128, D], F32, kind="ExternalOutput").ap()

    RG = [[0, 1, 2, 3], [4, 5, 6, 7]]
    EXP = mybir.ActivationFunctionType.Exp

    with tile.TileContext(nc) as tc:
        with (
            tc.tile_pool(name="wpool", bufs=1) as wpool,
            tc.tile_pool(name="xpool", bufs=12) as xpool,
            tc.tile_pool(name="big", bufs=4) as bigpool,
            tc.tile_pool(name="work", bufs=2) as work,
            tc.tile_pool(name="ppool", bufs=6) as ppool,
            tc.tile_pool(name="oopool", bufs=3) as oopool,
            tc.tile_pool(name="ps_acc", bufs=2, space="PSUM") as ps_acc,
            tc.tile_pool(name="ps_s", bufs=2, space="PSUM") as ps_s,
            tc.tile_pool(name="ps_o", bufs=2, space="PSUM") as ps_o,
            tc.tile_pool(name="dram", bufs=2, space="DRAM") as dram_pool,
        ):
            # ---- resident weights / tables -------------------------------
            wq_sb = wpool.tile([128, KC, CD], F32R, tag="wq")
            wk_sb = wpool.tile([128, KC, CD], F32R, tag="wk")
            wv_sb = wpool.tile([128, KC, CD], F32R, tag="wv")
            wo_sb = wpool.tile([128, 2, D], F32R, tag="wo")
            r2_sb = wpool.tile([128, 128], F32R, tag="r2")
            cos_sb = wpool.tile([128, S], F32, tag="cos")
            sin_sb = wpool.tile([128, S], F32, tag="sin")
            # x(T=0) and weight DMAs interleaved per-kc so the first
            # accumulation chain unblocks as early as possible
            xs0 = []
            for kc in range(KC):
                xt = xpool.tile([128, TB], F32R, tag="x", name=f"x0_{kc}")
                nc.sync.dma_start(xt[:], xT_d[kc * 128:(kc + 1) * 128, 0:TB])
                xs0.append(xt)
                nc.sync.dma_start(wq_sb[:, kc, :], wq_d[kc * 128:(kc + 1) * 128, :])
                nc.sync.dma_start(wk_sb[:, kc, :], wk_d[kc * 128:(kc + 1) * 128, :])
            nc.sync.dma_start(r2_sb[:], r2_d[:])
            nc.sync.dma_start(cos_sb[:], cos_d[:])
            nc.sync.dma_start(sin_sb[:], sin_d[:])
            for kc in range(KC):
                nc.sync.dma_start(wv_sb[:, kc, :], wv_d[kc * 128:(kc + 1) * 128, :])
            nc.sync.dma_start(wo_sb[:], wo_d.rearrange("(c p) n -> p c n", p=128))

            # ---- persistent per-block tensors (per rep) ------------------
            def alloc_rep(rep):
                sfx = f"_r{rep}" if rep else ""
                qr = [[bigpool.tile([128, TB], F32R, tag=f"qr{i}",
                                    name=f"qr{i}_{t}{sfx}")
                       for t in range(NT)] for i in range(2)]
                kr = [[bigpool.tile([128, TB], F32R, tag=f"kr{i}",
                                    name=f"kr{i}_{t}{sfx}")
                       for t in range(NT)] for i in range(2)]
                ot = [[bigpool.tile([128, TB], F32R, tag=f"ot{i}",
                                    name=f"ot{i}_{t}{sfx}")
                       for t in range(NT)] for i in range(2)]
                # v with ones column: [128 tok, 4 sub, 4 heads, 65]
                vb = [bigpool.tile([128, 4, HPC, 65], F32R, tag="vb",
                                   name=f"vb_{t}{sfx}")
                      for t in range(NT)]
                for T in range(NT):
                    nc.gpsimd.memset(vb[T][:, :, :, 64:65].bitcast(F32), 1.0)
                return qr, kr, ot, vb

            qr, kr, ot, vb = alloc_rep(0)

            def phase_a(T, xs=None):
                sfx = _sfx[0]
                T0 = T * TB
                if xs is None:
                    xs = []
                    for kc in range(KC):
                        xt = xpool.tile([128, TB], F32R, tag="x",
                                        name=f"x{T}_{kc}{sfx}")
                        nc.sync.dma_start(
                            xt[:], xT_d[kc * 128:(kc + 1) * 128, T0:T0 + TB])
                        xs.append(xt)
                for w_sb, dst in ((wq_sb, qr), (wk_sb, kr)):
                    for i in range(2):
                        acc = ps_acc.tile([128, TB], F32, tag="acc")
                        for kc in range(KC):
                            nc.tensor.matmul(
                                acc[:], lhsT=w_sb[:, kc, i * 128:(i + 1) * 128],
                                rhs=xs[kc][:], start=(kc == 0), stop=(kc == KC - 1))
                        raw = work.tile([128, TB], F32R, tag="raw")
                        nc.scalar.copy(raw[:], acc[:])
                        rot = ps_acc.tile([128, TB], F32, tag="acc", name=f"rot_{T}_{i}{sfx}")
                        nc.tensor.matmul(rot[:], lhsT=r2_sb[:], rhs=raw[:],
                                         start=True, stop=True)
                        t1 = work.tile([128, TB], F32, tag="t1")
                        nc.gpsimd.tensor_mul(t1[:], raw[:].bitcast(F32),
                                             cos_sb[:, T0:T0 + TB])
                        t2 = work.tile([128, TB], F32, tag="t2")
                        nc.vector.tensor_mul(t2[:], rot[:], sin_sb[:, T0:T0 + TB])
                        nc.gpsimd.tensor_add(dst[i][T][:], t1[:], t2[:])
                for j in range(4):
                    vp = ps_acc.tile([128, CD], F32, tag="acc", name=f"vp_{T}_{j}{sfx}")
                    for kc in range(KC):
                        nc.tensor.matmul(
                            vp[:], lhsT=xs[kc][:, j * 128:(j + 1) * 128],
                            rhs=wv_sb[:, kc, :], start=(kc == 0), stop=(kc == KC - 1))
                    nc.vector.tensor_copy(vb[T][:, j, :, 0:64],
                                          vp[:].rearrange("p (h d) -> p h d", h=HPC))

            def phase_b(T):
                sfx = _sfx[0]
                T0 = T * TB
                nch = 4 * (T + 1)
                for i in range(2):
                    o_ps = [ps_o.tile([65, TB], F32, tag="o", name=f"o_{T}_{i}_{s}{sfx}")
                            for s in range(2)]
                    for c in range(nch):
                        Tc, jj = divmod(c, 4)
                        # causal: keys in chunk c contribute nothing for
                        # tq < lo; [lo, lo+128) is the triangular boundary
                        diag = c >= 4 * T
                        lo = (c - 4 * T) * 128 if diag else 0
                        s_ps = ps_s.tile([128, 2, TB], F32, tag="s",
                                         name=f"s_{T}_{i}_{c}{sfx}")
                        for sh in range(2):
                            bp = sh * 64
                            nc.tensor.matmul(
                                s_ps[:, sh, :],
                                lhsT=kr[i][Tc][bp:bp + 64, jj * 128:(jj + 1) * 128],
                                rhs=qr[i][T][bp:bp + 64, :],
                                start=True, stop=True)
                        p = ppool.tile([128, 2, TB], F32R, tag="p")
                        nc.scalar.activation(p[:, :, lo:TB], s_ps[:, :, lo:TB], EXP)
                        if diag:
                            # zero stale [0,lo) plus the upper triangle of
                            # the boundary band: keep tq >= tk
                            for sh in range(2):
                                nc.gpsimd.affine_select(
                                    out=p[:, sh, 0:lo + 128],
                                    in_=p[:, sh, 0:lo + 128],
                                    compare_op=mybir.AluOpType.is_ge,
                                    fill=0.0, base=-lo,
                                    pattern=[[1, lo + 128]],
                                    channel_multiplier=-1)
                        for sh in range(2):
                            h = 2 * i + sh
                            nc.tensor.matmul(
                                o_ps[sh][:], lhsT=vb[Tc][:, jj, h, :],
                                rhs=p[:, sh, :],
                                start=(c == 0), stop=(c == nch - 1),
                                skip_group_check=True)
                    for sh in range(2):
                        bp = sh * 64
                        rr = work.tile([1, TB], F32, tag="rr")
                        nc.vector.reciprocal(rr[:], o_ps[sh][64:65, :])
                        bc = work.tile([64, TB], F32, tag="bc")
                        nc.gpsimd.partition_broadcast(bc[:], rr[:])
                        nc.vector.tensor_mul(ot[i][T][bp:bp + 64, :],
                                             o_ps[sh][0:64, :], bc[:])

            def phase_c(T):
                blk = dram_pool.tile([TB, D], F32, tag="blk")
                for j in range(4):
                    oo = oopool.tile([128, D], F32, tag="oo")
                    for nh in range(2):
                        op = ps_acc.tile([128, 512], F32, tag="acc")
                        for i2 in range(2):
                            nc.tensor.matmul(
                                op[:], lhsT=ot[i2][T][:, j * 128:(j + 1) * 128],
                                rhs=wo_sb[:, i2, nh * 512:(nh + 1) * 512],
                                start=(i2 == 0), stop=(i2 == 1))
                        if nh == 0:
                            nc.vector.tensor_copy(oo[:, 0:512], op[:])
                        else:
                            nc.scalar.copy(oo[:, 512:1024], op[:])
                    nc.sync.dma_start(blk[j * 128:(j + 1) * 128, :], oo[:])
                if collective:
                    rs = dram_pool.tile([128, D], F32, tag="rs")
                    nc.gpsimd.collective_compute(
                        "ReduceScatter", mybir.AluOpType.add, replica_groups=RG,
                        ins=[blk.opt()], outs=[rs.opt()])
                    nc.sync.dma_start(out_d[T], rs[:])
                else:
                    nc.sync.dma_start(out_d[T], blk[0:128, :])

            _sfx = [""]
            for rep in range(reps):
                if rep:
                    _sfx[0] = f"_r{rep}"
                    qr, kr, ot, vb = alloc_rep(rep)
                phase_a(0, xs=xs0 if rep == 0 else None)
                for T in range(NT):
                    if T + 1 < NT:
                        phase_a(T + 1)
                    phase_b(T)
                    phase_c(T)

    nc.compile()
    return nc


def _host_inputs(x, positions, wq, wk, wv, wo):
    x = np.asarray(x, dtype=np.float32)
    pos = np.asarray(positions).astype(np.float64)
    wq = np.asarray(wq, dtype=np.float32)
    wk = np.asarray(wk, dtype=np.float32)
    wv = np.asarray(wv, dtype=np.float32)
    wo = np.asarray(wo, dtype=np.float32)

    # RoPE tables in the transposed (row = dk index) layout, tiled to 2 heads
    inv = 1.0 / (THETA ** (np.arange(0, DK, 2, dtype=np.float64) / DK))
    fr = pos[:, None] * inv[None, :]            # (S, 32)
    cos = np.repeat(np.cos(fr), 2, axis=-1).T   # (64, S)
    sin = np.repeat(np.sin(fr), 2, axis=-1).T
    cosT = np.ascontiguousarray(np.tile(cos, (2, 1)), dtype=np.float32)
    sinT = np.ascontiguousarray(np.tile(sin, (2, 1)), dtype=np.float32)

    # interleaved rotate-half as a 64x64 permutation; lhsT = blockdiag(R, R).T
    R = np.zeros((DK, DK), np.float32)
    for r in range(DK // 2):
        R[2 * r, 2 * r + 1] = -1.0
        R[2 * r + 1, 2 * r] = 1.0
    r2T = np.zeros((128, 128), np.float32)
    r2T[0:64, 0:64] = R.T
    r2T[64:128, 64:128] = R.T

    xT = [np.ascontiguousarray(x[b].T) for b in range(B)]
    scale = np.float32(1.0 / np.sqrt(DK))
    wqT, wkT, wvT, woT = [], [], [], []
    for g in range(4):
        rows = slice(g * CD, (g + 1) * CD)
        wqT.append(np.ascontiguousarray((wq[rows] * scale).T))
        wkT.append(np.ascontiguousarray(wk[rows].T))
        wvT.append(np.ascontiguousarray(wv[rows].T))
        woT.append(np.ascontiguousarray(wo[:, rows].T))

    in_maps = []
    for c in range(NCORES):
        b, g = divmod(c, 4)
        in_maps.append({
            "xT": xT[b], "wqT": wqT[g], "wkT": wkT[g], "wvT": wvT[g],
            "woT": woT[g], "r2T": r2T, "cosT": cosT, "sinT": sinT,
        })
    return in_maps


def _make_runner(nc):
    """Build a cached PJRT executor for the SPMD kernel (mirrors
    bass2jax.run_bass_via_pjrt but reuses the jitted executable across
    calls)."""
    import jax
    import numpy as _np
    from jax.sharding import Mesh, PartitionSpec
    from jax.experimental.shard_map import shard_map
    import concourse.mybir as _mybir
    from concourse import bass2jax

    bass2jax.install_neuronx_cc_hook()

    in_names, out_names, out_avals, zero_shapes = [], [], [], []
    partition_name = (nc.partition_id_tensor.name
                      if nc.partition_id_tensor else None)
    for alloc in nc.m.functions[0].allocations:
        if not isinstance(alloc, _mybir.MemoryLocationSet):
            continue
        name = alloc.memorylocations[0].name
        if alloc.kind == "ExternalInput":
            if name != partition_name:
                in_names.append(name)
        elif alloc.kind == "ExternalOutput":
            out_names.append(name)
            shape = tuple(alloc.tensor_shape)
            dtype = _mybir.dt.np(alloc.dtype)
            out_avals.append(jax.core.ShapedArray(shape, dtype))
            zero_shapes.append((shape, dtype))
    n_params = len(in_names)
    n_outs = len(out_names)
    all_names = in_names + out_names
    if partition_name is not None:
        all_names.append(partition_name)
    donate = tuple(range(n_params, n_params + n_outs))

    def _body(*args):
        operands = list(args)
        if partition_name is not None:
            operands.append(bass2jax.partition_id_tensor())
        outs = bass2jax._bass_exec_p.bind(
            *operands,
            out_avals=tuple(out_avals),
            in_names=tuple(all_names),
            out_names=tuple(out_names),
            lowering_input_output_aliases=(),
            sim_require_finite=True,
            sim_require_nnan=True,
            nc=nc,
        )
        return tuple(outs)

    devices = jax.devices()[:NCORES]
    mesh = Mesh(_np.asarray(devices), ("core",))
    in_specs = (PartitionSpec("core"),) * (n_params + n_outs)
    out_specs = (PartitionSpec("core"),) * n_outs
    sharded = jax.jit(
        shard_map(_body, mesh=mesh, in_specs=in_specs, out_specs=out_specs,
                  check_rep=False),
        keep_unused=True)
    sharding = jax.sharding.NamedSharding(mesh, PartitionSpec("core"))

    def prepare(in_maps):
        concat_in = [
            _np.concatenate([_np.asarray(m[name]) for m in in_maps], axis=0)
            for name in in_names]
        concat_zeros = [
            _np.zeros((NCORES * s[0], *s[1:]), dt) for s, dt in zero_shapes]
        return [jax.device_put(a, sharding) for a in concat_in + concat_zeros]

    def execute(dev_args):
        out_arrs = sharded(*dev_args)
        jax.block_until_ready(out_arrs)
        return out_arrs

    def run(in_maps):
        out_arrs = execute(prepare(in_maps))
        return [
            {name: _np.asarray(out_arrs[i]).reshape(
                NCORES, *out_avals[i].shape)[c]
             for i, name in enumerate(out_names)}
            for c in range(NCORES)]

    run.prepare = prepare
    run.execute = execute
    return run


def _get_runner():
    if "run" not in _CACHE:
        nc = _build()
        _CACHE["nc"] = nc
        try:
            _CACHE["run"] = _make_runner(nc)
        except Exception:
            _CACHE["run"] = lambda in_maps: run_bass_kernel_spmd(
                nc, in_maps, list(range(NCORES))).results
    return _CACHE["run"]


def kernel(x, positions, wq, wk, wv, wo):
    run = _get_runner()
    in_maps = _host_inputs(x, positions, wq, wk, wv, wo)
    results = run(in_maps)
    out = np.empty((B, S, D), np.float32)
    for c in range(NCORES):
        b, r = divmod(c, 4)
        blk = results[c]["out"]              # (NT, 128, D)
        for T in range(NT):
            out[b, T * TB + r * 128: T * TB + (r + 1) * 128, :] = blk[T]
    return out


# revision 43
# speedup vs baseline: 1.3520x; 1.3520x over previous
"""Trainium2 Bass kernel for 16-head causal MHA with interleaved RoPE.

Problem: x (2, 2048, 1024) f32, wq/wk/wv/wo (1024, 1024) f32, positions arange(2048).
  q/k/v = x @ W.T ; RoPE(q, k) ; causal softmax attention ; out @ wo.T

Sharding: hybrid batch x head tensor-parallel over 8 cores.
  core c -> batch b = c//4, head group g = c%4 (4 heads = 256 of the 1024 c-dim).
  Each core computes its 4 heads' attention output and a partial o_proj
  (2048, 1024); a ReduceScatter over the 4 cores of each batch sums partials
  and leaves each core with a distinct 512-row slice, reassembled on host.

Device layout (per core):
  qT/kT computed transposed: (256 rows = 4 heads x 64 dk, 2048 tokens), RoPE
  applied in this layout via a PE permutation matmul + DVE/GPSIMD elementwise.
  v natural (2048, 4, 65) with a ones column -> softmax denominator comes out
  of the attention-value matmul (row 64 of each per-head PSUM accumulator).
  Scores computed transposed (tk partition, tq free) per 128-key chunk;
  exp on ACT; causal masking by zeroing post-exp via gpsimd.affine_select.
  Matmul operands are float32r (TF32-like, 1 PE cycle/row vs 4 for fp32).
"""

import sys

sys.path.insert(0, "/opt/trn_rl_repo")

import numpy as np

import concourse.bacc as bacc
import concourse.tile as tile
import concourse.mybir as mybir
from concourse.bass_utils import run_bass_kernel_spmd

B, S, D = 2, 2048, 1024
H, DK = 16, 64
NCORES = 8
HPC = 4              # heads per core
CD = HPC * DK        # 256: local c-dim (slice of D owned by one core)
TB = 512             # tq block size
NT = S // TB         # 4 query blocks
KC = D // 128        # 8 contraction chunks for the projections
THETA = 10000.0

F32 = mybir.dt.float32
F32R = mybir.dt.float32r

_CACHE = {}


def _build(collective=True, reps=1):
    nc = bacc.Bacc("TRN2", target_bir_lowering=False, debug=False,
                   num_devices=NCORES if collective else 1)

    xT_d = nc.dram_tensor("xT", [D, S], F32R, kind="ExternalInput").ap()
    wq_d = nc.dram_tensor("wqT", [D, CD], F32R, kind="ExternalInput").ap()
    wk_d = nc.dram_tensor("wkT", [D, CD], F32R, kind="ExternalInput").ap()
    wv_d = nc.dram_tensor("wvT", [D, CD], F32R, kind="ExternalInput").ap()
    wo_d = nc.dram_tensor("woT", [CD, D], F32R, kind="ExternalInput").ap()
    r2_d = nc.dram_tensor("r2T", [128, 128], F32R, kind="ExternalInput").ap()
    cos_d = nc.dram_tensor("cosT", [128, S], F32, kind="ExternalInput").ap()
    sin_d = nc.dram_tensor("sinT", [128, S], F32, kind="ExternalInput").ap()
    out_d = nc.dram_tensor("out", [NT, 128, D], F32, kind="ExternalOutput").ap()

    RG = [[0, 1, 2, 3], [4, 5, 6, 7]]
    EXP = mybir.ActivationFunctionType.Exp

    with tile.TileContext(nc) as tc:
        with (
            tc.tile_pool(name="wpool", bufs=1) as wpool,
            tc.tile_pool(name="xpool", bufs=14) as xpool,
            tc.tile_pool(name="big", bufs=4) as bigpool,
            tc.tile_pool(name="work", bufs=2) as work,
            tc.tile_pool(name="ppool", bufs=6) as ppool,
            tc.tile_pool(name="oopool", bufs=4) as oopool,
            tc.tile_pool(name="ps_acc", bufs=2, space="PSUM") as ps_acc,
            tc.tile_pool(name="ps_s", bufs=2, space="PSUM") as ps_s,
            tc.tile_pool(name="ps_o", bufs=2, space="PSUM") as ps_o,
            tc.tile_pool(name="dram", bufs=4, space="DRAM") as dram_pool,
        ):
            # ---- resident weights / tables -------------------------------
            wq_sb = wpool.tile([128, KC, CD], F32R, tag="wq")
            wk_sb = wpool.tile([128, KC, CD], F32R, tag="wk")
            wv_sb = wpool.tile([128, KC, CD], F32R, tag="wv")
            wo_sb = wpool.tile([128, 2, D], F32R, tag="wo")
            r2_sb = wpool.tile([128, 128], F32R, tag="r2")
            cos_sb = wpool.tile([128, S], F32, tag="cos")
            sin_sb = wpool.tile([128, S], F32, tag="sin")
            # x(T=0) and weight DMAs interleaved per-kc so the first
            # accumulation chain unblocks as early as possible
            xs0 = []
            for kc in range(KC):
                xt = xpool.tile([128, TB], F32R, tag="x", name=f"x0_{kc}")
                nc.sync.dma_start(xt[:], xT_d[kc * 128:(kc + 1) * 128, 0:TB])
                xs0.append(xt)
                nc.sync.dma_start(wq_sb[:, kc, :], wq_d[kc * 128:(kc + 1) * 128, :])
                nc.sync.dma_start(wk_sb[:, kc, :], wk_d[kc * 128:(kc + 1) * 128, :])
            nc.sync.dma_start(r2_sb[:], r2_d[:])
            nc.sync.dma_start(cos_sb[:], cos_d[:])
            nc.sync.dma_start(sin_sb[:], sin_d[:])
            for kc in range(KC):
                nc.sync.dma_start(wv_sb[:, kc, :], wv_d[kc * 128:(kc + 1) * 128, :])
            nc.sync.dma_start(wo_sb[:], wo_d.rearrange("(c p) n -> p c n", p=128))

            # ---- persistent per-block tensors (per rep) ------------------
            def alloc_rep(rep):
                sfx = f"_r{rep}" if rep else ""
                qr = [[bigpool.tile([128, TB], F32R, tag=f"qr{i}",
                                    name=f"qr{i}_{t}{sfx}")
                       for t in range(NT)] for i in range(2)]
                kr = [[bigpool.tile([128, TB], F32R, tag=f"kr{i}",
                                    name=f"kr{i}_{t}{sfx}")
                       for t in range(NT)] for i in range(2)]
                ot = [[bigpool.tile([128, TB], F32R, tag=f"ot{i}",
                                    name=f"ot{i}_{t}{sfx}")
                       for t in range(NT)] for i in range(2)]
                # v with ones column: [128 tok, 4 sub, 4 heads, 65]
                vb = [bigpool.tile([128, 4, HPC, 65], F32R, tag="vb",
                                   name=f"vb_{t}{sfx}")
                      for t in range(NT)]
                for T in range(NT):
                    nc.gpsimd.memset(vb[T][:, :, :, 64:65].bitcast(F32), 1.0)
                return qr, kr, ot, vb

            qr, kr, ot, vb = alloc_rep(0)

            def phase_a(T, xs=None):
                sfx = _sfx[0]
                T0 = T * TB
                if xs is None:
                    xs = []
                    for kc in range(KC):
                        xt = xpool.tile([128, TB], F32R, tag="x",
                                        name=f"x{T}_{kc}{sfx}")
                        nc.sync.dma_start(
                            xt[:], xT_d[kc * 128:(kc + 1) * 128, T0:T0 + TB])
                        xs.append(xt)
                for w_sb, dst in ((wq_sb, qr), (wk_sb, kr)):
                    for i in range(2):
                        acc = ps_acc.tile([128, TB], F32, tag="acc")
                        for kc in range(KC):
                            nc.tensor.matmul(
                                acc[:], lhsT=w_sb[:, kc, i * 128:(i + 1) * 128],
                                rhs=xs[kc][:], start=(kc == 0), stop=(kc == KC - 1))
                        raw = work.tile([128, TB], F32R, tag="raw")
                        nc.scalar.copy(raw[:], acc[:])
                        rot = ps_acc.tile([128, TB], F32, tag="acc", name=f"rot_{T}_{i}{sfx}")
                        nc.tensor.matmul(rot[:], lhsT=r2_sb[:], rhs=raw[:],
                                         start=True, stop=True)
                        t1 = work.tile([128, TB], F32, tag="t1")
                        nc.gpsimd.tensor_mul(t1[:], raw[:].bitcast(F32),
                                             cos_sb[:, T0:T0 + TB])
                        t2 = work.tile([128, TB], F32, tag="t2")
                        nc.vector.tensor_mul(t2[:], rot[:], sin_sb[:, T0:T0 + TB])
                        nc.gpsimd.tensor_add(dst[i][T][:], t1[:], t2[:])
                for j in range(4):
                    vp = ps_acc.tile([128, CD], F32, tag="acc", name=f"vp_{T}_{j}{sfx}")
                    for kc in range(KC):
                        nc.tensor.matmul(
                            vp[:], lhsT=xs[kc][:, j * 128:(j + 1) * 128],
                            rhs=wv_sb[:, kc, :], start=(kc == 0), stop=(kc == KC - 1))
                    nc.vector.tensor_copy(vb[T][:, j, :, 0:64],
                                          vp[:].rearrange("p (h d) -> p h d", h=HPC))

            def phase_b(T):
                sfx = _sfx[0]
                T0 = T * TB
                nch = 4 * (T + 1)
                for i in range(2):
                    o_ps = [ps_o.tile([65, TB], F32, tag="o", name=f"o_{T}_{i}_{s}{sfx}")
                            for s in range(2)]
                    for c in range(nch):
                        Tc, jj = divmod(c, 4)
                        # causal: keys in chunk c contribute nothing for
                        # tq < lo; [lo, lo+128) is the triangular boundary
                        diag = c >= 4 * T
                        lo = (c - 4 * T) * 128 if diag else 0
                        s_ps = ps_s.tile([128, 2, TB], F32, tag="s",
                                         name=f"s_{T}_{i}_{c}{sfx}")
                        for sh in range(2):
                            bp = sh * 64
                            nc.tensor.matmul(
                                s_ps[:, sh, :],
                                lhsT=kr[i][Tc][bp:bp + 64, jj * 128:(jj + 1) * 128],
                                rhs=qr[i][T][bp:bp + 64, :],
                                start=True, stop=True)
                        p = ppool.tile([128, 2, TB], F32R, tag="p")
                        nc.scalar.activation(p[:, :, lo:TB], s_ps[:, :, lo:TB], EXP)
                        if diag:
                            # zero stale [0,lo) plus the upper triangle of
                            # the boundary band: keep tq >= tk
                            for sh in range(2):
                                nc.gpsimd.affine_select(
                                    out=p[:, sh, 0:lo + 128],
                                    in_=p[:, sh, 0:lo + 128],
                                    compare_op=mybir.AluOpType.is_ge,
                                    fill=0.0, base=-lo,
                                    pattern=[[1, lo + 128]],
                                    channel_multiplier=-1)
                        for sh in range(2):
                            h = 2 * i + sh
                            nc.tensor.matmul(
                                o_ps[sh][:], lhsT=vb[Tc][:, jj, h, :],
                                rhs=p[:, sh, :],
                                start=(c == 0), stop=(c == nch - 1),
                                skip_group_check=True)
                    for sh in range(2):
                        bp = sh * 64
                        rr = work.tile([1, TB], F32, tag="rr")
                        nc.vector.reciprocal(rr[:], o_ps[sh][64:65, :])
                        bc = work.tile([64, TB], F32, tag="bc")
                        nc.gpsimd.partition_broadcast(bc[:], rr[:])
                        nc.vector.tensor_mul(ot[i][T][bp:bp + 64, :],
                                             o_ps[sh][0:64, :], bc[:])

            def phase_c(T):
                blk = dram_pool.tile([TB, D], F32, tag="blk")
                for j in range(4):
                    oo = oopool.tile([128, D], F32, tag="oo")
                    for nh in range(2):
                        op = ps_acc.tile([128, 512], F32, tag="acc")
                        for i2 in range(2):
                            nc.tensor.matmul(
                                op[:], lhsT=ot[i2][T][:, j * 128:(j + 1) * 128],
                                rhs=wo_sb[:, i2, nh * 512:(nh + 1) * 512],
                                start=(i2 == 0), stop=(i2 == 1))
                        if nh == 0:
                            nc.vector.tensor_copy(oo[:, 0:512], op[:])
                        else:
                            nc.scalar.copy(oo[:, 512:1024], op[:])
                    nc.sync.dma_start(blk[j * 128:(j + 1) * 128, :], oo[:])
                if collective:
                    rs = dram_pool.tile([128, D], F32, tag="rs")
                    nc.gpsimd.collective_compute(
                        "ReduceScatter", mybir.AluOpType.add, replica_groups=RG,
                        ins=[blk.opt()], outs=[rs.opt()])
                    nc.sync.dma_start(out_d[T], rs[:])
                else:
                    nc.sync.dma_start(out_d[T], blk[0:128, :])

            _sfx = [""]
            for rep in range(reps):
                if rep:
                    _sfx[0] = f"_r{rep}"
                    qr, kr, ot, vb = alloc_rep(rep)
                phase_a(0, xs=xs0 if rep == 0 else None)
                for T in range(NT):
                    if T + 1 < NT:
                        phase_a(T + 1)
                    phase_b(T)
                    phase_c(T)

    nc.compile()
    return nc


def _host_inputs(x, positions, wq, wk, wv, wo):
    x = np.asarray(x, dtype=np.float32)
    pos = np.asarray(positions).astype(np.float64)
    wq = np.asarray(wq, dtype=np.float32)
    wk = np.asarray(wk, dtype=np.float32)
    wv = np.asarray(wv, dtype=np.float32)
    wo = np.asarray(wo, dtype=np.float32)

    # RoPE tables in the transposed (row = dk index) layout, tiled to 2 heads
    inv = 1.0 / (THETA ** (np.arange(0, DK, 2, dtype=np.float64) / DK))
    fr = pos[:, None] * inv[None, :]            # (S, 32)
    cos = np.repeat(np.cos(fr), 2, axis=-1).T   # (64, S)
    sin = np.repeat(np.sin(fr), 2, axis=-1).T
    cosT = np.ascontiguousarray(np.tile(cos, (2, 1)), dtype=np.float32)
    sinT = np.ascontiguousarray(np.tile(sin, (2, 1)), dtype=np.float32)

    # interleaved rotate-half as a 64x64 permutation; lhsT = blockdiag(R, R).T
    R = np.zeros((DK, DK), np.float32)
    for r in range(DK // 2):
        R[2 * r, 2 * r + 1] = -1.0
        R[2 * r + 1, 2 * r] = 1.0
    r2T = np.zeros((128, 128), np.float32)
    r2T[0:64, 0:64] = R.T
    r2T[64:128, 64:128] = R.T

    xT = [np.ascontiguousarray(x[b].T) for b in range(B)]
    scale = np.float32(1.0 / np.sqrt(DK))
    wqT, wkT, wvT, woT = [], [], [], []
    for g in range(4):
        rows = slice(g * CD, (g + 1) * CD)
        wqT.append(np.ascontiguousarray((wq[rows] * scale).T))
        wkT.append(np.ascontiguousarray(wk[rows].T))
        wvT.append(np.ascontiguousarray(wv[rows].T))
        woT.append(np.ascontiguousarray(wo[:, rows].T))

    in_maps = []
    for c in range(NCORES):
        b, g = divmod(c, 4)
        in_maps.append({
            "xT": xT[b], "wqT": wqT[g], "wkT": wkT[g], "wvT": wvT[g],
            "woT": woT[g], "r2T": r2T, "cosT": cosT, "sinT": sinT,
        })
    return in_maps


def _make_runner(nc):
    """Build a cached PJRT executor for the SPMD kernel (mirrors
    bass2jax.run_bass_via_pjrt but reuses the jitted executable across
    calls)."""
    import jax
    import numpy as _np
    from jax.sharding import Mesh, PartitionSpec
    from jax.experimental.shard_map import shard_map
    import concourse.mybir as _mybir
    from concourse import bass2jax

    bass2jax.install_neuronx_cc_hook()

    in_names, out_names, out_avals, zero_shapes = [], [], [], []
    partition_name = (nc.partition_id_tensor.name
                      if nc.partition_id_tensor else None)
    for alloc in nc.m.functions[0].allocations:
        if not isinstance(alloc, _mybir.MemoryLocationSet):
            continue
        name = alloc.memorylocations[0].name
        if alloc.kind == "ExternalInput":
            if name != partition_name:
                in_names.append(name)
        elif alloc.kind == "ExternalOutput":
            out_names.append(name)
            shape = tuple(alloc.tensor_shape)
            dtype = _mybir.dt.np(alloc.dtype)
            out_avals.append(jax.core.ShapedArray(shape, dtype))
            zero_shapes.append((shape, dtype))
    n_params = len(in_names)
    n_outs = len(out_names)
    all_names = in_names + out_names
    if partition_name is not None:
        all_names.append(partition_name)
    donate = tuple(range(n_params, n_params + n_outs))

    def _body(*args):
        operands = list(args)
        if partition_name is not None:
            operands.append(bass2jax.partition_id_tensor())
        outs = bass2jax._bass_exec_p.bind(
            *operands,
            out_avals=tuple(out_avals),
            in_names=tuple(all_names),
            out_names=tuple(out_names),
            lowering_input_output_aliases=(),
            sim_require_finite=True,
            sim_require_nnan=True,
            nc=nc,
        )
        return tuple(outs)

    devices = jax.devices()[:NCORES]
    mesh = Mesh(_np.asarray(devices), ("core",))
    in_specs = (PartitionSpec("core"),) * (n_params + n_outs)
    out_specs = (PartitionSpec("core"),) * n_outs
    sharded = jax.jit(
        shard_map(_body, mesh=mesh, in_specs=in_specs, out_specs=out_specs,
                  check_rep=False),
        keep_unused=True)
    sharding = jax.sharding.NamedSharding(mesh, PartitionSpec("core"))

    def prepare(in_maps):
        concat_in = [
            _np.concatenate([_np.asarray(m[name]) for m in in_maps], axis=0)
            for name in in_names]
        concat_zeros = [
            _np.zeros((NCORES * s[0], *s[1:]), dt) for s, dt in zero_shapes]
        return [jax.device_put(a, sharding) for a in concat_in + concat_zeros]

    def execute(dev_args):
        out_arrs = sharded(*dev_args)
        jax.block_until_ready(out_arrs)
        return out_arrs

    def run(in_maps):
        out_arrs = execute(prepare(in_maps))
        return [
            {name: _np.asarray(out_arrs[i]).reshape(
                NCORES, *out_avals[i].shape)[c]
             for i, name in enumerate(out_names)}
            for c in range(NCORES)]

    run.prepare = prepare
    run.execute = execute
    return run


def _get_runner():
    if "run" not in _CACHE:
        nc = _build()
        _CACHE["nc"] = nc
        try:
            _CACHE["run"] = _make_runner(nc)
        except Exception:
            _CACHE["run"] = lambda in_maps: run_bass_kernel_spmd(
                nc, in_maps, list(range(NCORES))).results
    return _CACHE["run"]


def kernel(x, positions, wq, wk, wv, wo):
    run = _get_runner()
    in_maps = _host_inputs(x, positions, wq, wk, wv, wo)
    results = run(in_maps)
    out = np.empty((B, S, D), np.float32)
    for c in range(NCORES):
        b, r = divmod(c, 4)
        blk = results[c]["out"]              # (NT, 128, D)
        for T in range(NT):
            out[b, T * TB + r * 128: T * TB + (r + 1) * 128, :] = blk[T]
    return out
